# revision 18
# baseline (speedup 1.0000x reference)
"""Trainium2 Bass kernel for nn_MemoryUpdate (gated LIF memory update).

Reference computation (fp32):
    k         = einsum('tbnd,od->tbno', kv, Wg)          # kv @ Wg^T
    gate_mean = mean_t'( k[t', b, nkv, d] )              # [Nkv, B, 1, D], Nkv == T
    update    = gate_mean[t, b, d] * q[t, b, n, d]       # broadcast over n
    spikes    = LIF over t: v' = (v + u)/2 ; s = v' >= 0.5 ; v = v' * (1 - s)

Shapes: q [4, 32, 1024, 512], kv [4, 32, 4, 512], Wg [512, 512] -> out [4, 32, 1024, 512].

Strategy: data-parallel over B across 8 cores (B_loc = 4). Per core:
  - tiny on-device matmul for the gate:   g'[b*4+t, o] = sum_d kvsumT[d, b*4+t] * WgT[d, o]
    where WgT is host-side-transposed Wg scaled by 1/(2T)  (folds the mean and the
    LIF /tau into the gate), and kvsumT is the T-sum of host-transposed kv.
  - gate rows broadcast to [128, 2048] tiles via PE outer-product (ones ⊗ g_row).
  - main stream, tiles [128 part = n/4, free = (n_sub 4, d 512)] (8 KiB contiguous
    per partition per DMA):
        c = q * g_bc                  (DVE tensor_tensor)
        a = (w*0.5) + c               (DVE scalar_tensor_tensor; skipped at t=0)
        s = (a >= 0.5)                (GpSimd tensor_scalar)  -> store
        w = (a < 0.5) * a             (DVE scalar_tensor_tensor; skipped at last t)
Recurrence over t kept in SBUF; (b, n)-independent => fully pipelined.
"""

import sys

for p in ("/opt/trn_rl_repo", "/root/.axon_site/_ro/trn_rl_repo"):
    if p not in sys.path:
        sys.path.insert(0, p)

import numpy as np

import concourse.bass as bass
import concourse.mybir as mybir
import concourse.tile as tile
from concourse import bacc
from concourse.bass_utils import run_bass_kernel_spmd

# Problem constants (hardcoded per harness contract)
T, B, NQ, NKV, D = 4, 32, 1024, 4, 512
N_CORES = 8
B_LOC = B // N_CORES  # 4
V_TH = 0.5
NSUB = 4              # n-values packed per partition row
P = 128               # partitions
FREE = NSUB * D       # 2048
NH = NQ // (P * NSUB)  # 2 tile groups per (t, b)

FP32 = mybir.dt.float32
Alu = mybir.AluOpType
UNROLL = 8  # static inner unroll inside the timing-mode For_i loop


def build_kernel(repeats=1, timing_mode=False):
    # Bacc (not raw Bass): its compile() legalizes multi-sem waits, which the
    # walrus CoreV3 codegen can't carry on a single compute instruction.
    nc = bacc.Bacc("TRN2", target_bir_lowering=False, debug=False,
                   num_devices=N_CORES)

    if timing_mode:
        # timing-only variant: big tensors live in internal DRAM so the wall
        # clock isn't dominated by host<->device transfers; the main body runs
        # `repeats` times in an on-device loop.
        q = nc.dram_tensor("q_int", [T, B_LOC, NQ, D], FP32).ap()
        out = nc.dram_tensor("out_int", [T, B_LOC, NQ, D], FP32).ap()
        dummy = nc.dram_tensor("tiny_out", [P, 16], FP32, kind="ExternalOutput").ap()
    else:
        q = nc.dram_tensor("q", [T, B_LOC, NQ, D], FP32, kind="ExternalInput").ap()
        out = nc.dram_tensor("out", [T, B_LOC, NQ, D], FP32, kind="ExternalOutput").ap()
        dummy = None
    kvT = nc.dram_tensor("kvT", [D, T * B_LOC * NKV], FP32, kind="ExternalInput").ap()
    wgT = nc.dram_tensor("wgT", [D, D], FP32, kind="ExternalInput").ap()

    # [T, B, (nh p ns), d] -> [t, b, nh, p, (ns d)]
    q_v = q.rearrange("t b (nh p ns) d -> t b nh p (ns d)", nh=NH, p=P, ns=NSUB)
    out_v = out.rearrange("t b (nh p ns) d -> t b nh p (ns d)", nh=NH, p=P, ns=NSUB)
    # kvT rows: d = c*128 + p ; cols: i = t'*16 + b*4 + n
    kvT_v = kvT.rearrange("(c p) i -> p c i", p=P)
    wgT_v = wgT.rearrange("(c p) o -> p c o", p=P)
    NI = T * B_LOC * NKV  # 64
    NG = B_LOC * NKV      # 16 gate rows

    with tile.TileContext(nc) as tc:
        with (
            tc.tile_pool(name="const", bufs=1) as const_pool,
            tc.tile_pool(name="gbc", bufs=6) as gbc_pool,
            tc.tile_pool(name="qp", bufs=4) as q_pool,
            tc.tile_pool(name="cp", bufs=2) as c_pool,
            tc.tile_pool(name="ap", bufs=2) as a_pool,
            tc.tile_pool(name="wp", bufs=2) as w_pool,
            tc.tile_pool(name="sp", bufs=4) as s_pool,
            tc.tile_pool(name="psg", bufs=1, space="PSUM") as psg_pool,
            tc.tile_pool(name="gdram", bufs=1, space="DRAM") as gdram_pool,
        ):
            # ---- gate computation ----
            kvT_sb = const_pool.tile([P, 4 * NI], FP32, tag="kvT")
            nc.sync.dma_start(kvT_sb[:].rearrange("p (c i) -> p c i", c=4), kvT_v)
            wgT_sb = const_pool.tile([P, 4 * D], FP32, tag="wgT")
            nc.sync.dma_start(wgT_sb[:].rearrange("p (c o) -> p c o", c=4), wgT_v)

            # sum over t' of kvT (free layout per chunk: i = t'*16 + (b*4+n))
            kv4 = kvT_sb[:].rearrange("p (c tp i) -> p c tp i", c=4, tp=T)
            t01 = const_pool.tile([P, 4 * NG], FP32, tag="t01")
            t23 = const_pool.tile([P, 4 * NG], FP32, tag="t23")
            kvs = const_pool.tile([P, 4 * NG], FP32, tag="kvs")
            t01v = t01[:].rearrange("p (c i) -> p c i", c=4)
            t23v = t23[:].rearrange("p (c i) -> p c i", c=4)
            nc.vector.tensor_tensor(t01v, kv4[:, :, 0, :], kv4[:, :, 1, :], Alu.add)
            nc.vector.tensor_tensor(t23v, kv4[:, :, 2, :], kv4[:, :, 3, :], Alu.add)
            nc.vector.tensor_tensor(
                kvs[:].rearrange("p (c i) -> p c i", c=4), t01v, t23v, Alu.add
            )
            kvs_v = kvs[:].rearrange("p (c i) -> p c i", c=4)

            # scale Wg^T by 1/(2T) on DVE. Besides folding the mean and /tau into
            # the gate, this funnels both matmul operands through the DVE clock so
            # the PE load-weights slot needs only one sync wait (walrus limit).
            wgs = const_pool.tile([P, 4 * D], FP32, tag="wgs")
            nc.vector.tensor_scalar(
                wgs[:], wgT_sb[:], float(1.0 / (2.0 * T)), None, Alu.mult
            )
            wg_v = wgs[:].rearrange("p (c o) -> p c o", c=4)

            psum_g = psg_pool.tile([NG, D], FP32)
            for c in range(4):
                nc.tensor.matmul(psum_g[:], kvs_v[:, c, :], wg_v[:, c, :],
                                 start=(c == 0), stop=(c == 3))
            g_sb = const_pool.tile([NG, D], FP32, tag="gsb")
            nc.scalar.copy(g_sb[:], psum_g[:])

            # round-trip the gate through DRAM so it can be partition-broadcast
            # on reload (engines cannot broadcast across partitions; DMA can).
            g_dram = gdram_pool.tile([NG, D], FP32, tag="gdram")
            nc.sync.dma_start(g_dram[:], g_sb[:])

            if timing_mode:
                # fill internal q with a constant so timing needs no transfer
                qfill = const_pool.tile([P, FREE], FP32, tag="qfill")
                nc.vector.memset(qfill[:], 0.3)
                for t in range(T):
                    for b in range(B_LOC):
                        for nh in range(NH):
                            nc.sync.dma_start(q_v[t, b, nh], qfill[:])
                nc.sync.dma_start(dummy, qfill[:, :16])  # satisfy external output

            import contextlib
            if timing_mode and repeats > 1:
                assert repeats % UNROLL == 0
                rep_ctx = tc.For_i(0, repeats // UNROLL, 1)
                inner_reps = UNROLL
            else:
                rep_ctx = contextlib.nullcontext()
                inner_reps = 1

            # ---- main loop ----
            with rep_ctx:
             for _inner in range(inner_reps):
              for b in range(B_LOC):
                # gate rows for this b, broadcast to all 128 partitions
                gbc = []
                for t in range(T):
                    row = b * NKV + t
                    gt = gbc_pool.tile([P, D], FP32, tag="gbc")
                    nc.sync.dma_start(
                        gt[:], g_dram[row:row + 1, :].partition_broadcast(P)
                    )
                    gbc.append(gt)

                for nh in range(NH):
                    w_prev = None
                    for t in range(T):
                        qt = q_pool.tile([P, FREE], FP32, tag="q")
                        nc.sync.dma_start(qt[:], q_v[t, b, nh])
                        ct = c_pool.tile([P, FREE], FP32, tag="c")
                        g_rep = gbc[t][:, None, :].to_broadcast((P, NSUB, D))
                        nc.vector.tensor_tensor(
                            ct[:].rearrange("p (ns d) -> p ns d", ns=NSUB),
                            qt[:].rearrange("p (ns d) -> p ns d", ns=NSUB),
                            g_rep, Alu.mult,
                        )
                        if t == 0:
                            at = ct
                        else:
                            at = a_pool.tile([P, FREE], FP32, tag="a")
                            nc.vector.scalar_tensor_tensor(
                                at[:], w_prev[:], 0.5, ct[:], Alu.mult, Alu.add
                            )
                        st = s_pool.tile([P, FREE], FP32, tag="s")
                        nc.gpsimd.tensor_scalar(
                            st[:], at[:], V_TH, None, Alu.is_ge
                        )
                        nc.scalar.dma_start(out_v[t, b, nh], st[:])
                        if t < T - 1:
                            wt = w_pool.tile([P, FREE], FP32, tag="w")
                            nc.vector.scalar_tensor_tensor(
                                wt[:], at[:], V_TH, at[:], Alu.is_lt, Alu.mult
                            )
                            w_prev = wt
    nc.compile()
    return nc


_CACHED_NC = None


def _make_in_maps(q, kv, Wg):
    q = np.ascontiguousarray(q, dtype=np.float32)
    kv = np.ascontiguousarray(kv, dtype=np.float32)
    Wg = np.ascontiguousarray(Wg, dtype=np.float32)

    # transposed so the contraction dim lands on partitions; the 1/(2T) gate
    # scaling (mean over T + LIF /tau) is applied on-device
    wgT = np.ascontiguousarray(Wg.T)

    in_maps = []
    for i in range(N_CORES):
        b0 = i * B_LOC
        q_i = np.ascontiguousarray(q[:, b0:b0 + B_LOC])
        kv_i = kv[:, b0:b0 + B_LOC]  # [T, B_LOC, NKV, D]
        kvT_i = np.ascontiguousarray(
            kv_i.transpose(3, 0, 1, 2).reshape(D, T * B_LOC * NKV)
        )
        in_maps.append({"q": q_i, "kvT": kvT_i, "wgT": wgT})
    return in_maps


def kernel(q: np.ndarray, kv: np.ndarray, Wg: np.ndarray) -> np.ndarray:
    global _CACHED_NC
    if _CACHED_NC is None:
        _CACHED_NC = build_kernel()
    nc = _CACHED_NC

    in_maps = _make_in_maps(q, kv, Wg)
    res = run_bass_kernel_spmd(nc, in_maps, core_ids=list(range(N_CORES)))
    out = np.concatenate([r["out"] for r in res.results], axis=1)
    return out


if __name__ == "__main__":
    rng = np.random.default_rng(0)
    q = rng.standard_normal((T, B, NQ, D), dtype=np.float32)
    kv = rng.standard_normal((T, B, NKV, D), dtype=np.float32)
    Wg = (rng.standard_normal((D, D), dtype=np.float32) / np.sqrt(D)).astype(np.float32)
    o = kernel(q, kv, Wg)
    print("out", o.shape, o.dtype, "mean", o.mean())


# revision 22
# speedup vs baseline: 4.6566x; 4.6566x over previous
"""Trainium2 Bass kernel for nn_MemoryUpdate (gated LIF memory update).

Reference computation (fp32):
    k         = einsum('tbnd,od->tbno', kv, Wg)          # kv @ Wg^T
    gate_mean = mean_t'( k[t', b, nkv, d] )              # [Nkv, B, 1, D], Nkv == T
    update    = gate_mean[t, b, d] * q[t, b, n, d]       # broadcast over n
    spikes    = LIF over t: v' = (v + u)/2 ; s = v' >= 0.5 ; v = v' * (1 - s)

Shapes: q [4, 32, 1024, 512], kv [4, 32, 4, 512], Wg [512, 512] -> out [4, 32, 1024, 512].

Strategy: data-parallel over B across 8 cores (B_loc = 4). Per core:
  - tiny on-device matmul for the gate:   g'[b*4+t, o] = sum_d kvsumT[d, b*4+t] * WgT[d, o]
    where WgT is host-side-transposed Wg scaled by 1/(2T)  (folds the mean and the
    LIF /tau into the gate), and kvsumT is the T-sum of host-transposed kv.
  - gate rows broadcast to [128, 2048] tiles via PE outer-product (ones ⊗ g_row).
  - main stream, tiles [128 part = n/4, free = (n_sub 4, d 512)] (8 KiB contiguous
    per partition per DMA):
        c = q * g_bc                  (DVE tensor_tensor)
        a = (w*0.5) + c               (DVE scalar_tensor_tensor; skipped at t=0)
        s = (a >= 0.5)                (GpSimd tensor_scalar)  -> store
        w = (a < 0.5) * a             (DVE scalar_tensor_tensor; skipped at last t)
Recurrence over t kept in SBUF; (b, n)-independent => fully pipelined.
"""

import sys

for p in ("/opt/trn_rl_repo", "/root/.axon_site/_ro/trn_rl_repo"):
    if p not in sys.path:
        sys.path.insert(0, p)

import numpy as np

import concourse.bass as bass
import concourse.mybir as mybir
import concourse.tile as tile
from concourse import bacc
from concourse.bass_utils import run_bass_kernel_spmd

# Problem constants (hardcoded per harness contract)
T, B, NQ, NKV, D = 4, 32, 1024, 4, 512
N_CORES = 8
B_LOC = B // N_CORES  # 4
V_TH = 0.5
NSUB = 4              # n-values packed per partition row
P = 128               # partitions
FREE = NSUB * D       # 2048
NH = NQ // (P * NSUB)  # 2 tile groups per (t, b)

FP32 = mybir.dt.float32
Alu = mybir.AluOpType
UNROLL = 8  # static inner unroll inside the timing-mode For_i loop
_BIG = 1.0e30  # threshold-comparison scale; saturates sigmoid to exact 0/1


def build_kernel(repeats=1, timing_mode=False):
    # Bacc (not raw Bass): its compile() legalizes multi-sem waits, which the
    # walrus CoreV3 codegen can't carry on a single compute instruction.
    nc = bacc.Bacc("TRN2", target_bir_lowering=False, debug=False,
                   num_devices=N_CORES)

    if timing_mode:
        # timing-only variant: big tensors live in internal DRAM so the wall
        # clock isn't dominated by host<->device transfers; the main body runs
        # `repeats` times in an on-device loop.
        q = nc.dram_tensor("q_int", [T, B_LOC, NQ, D], FP32).ap()
        out = nc.dram_tensor("out_int", [T, B_LOC, NQ, D], FP32).ap()
        dummy = nc.dram_tensor("tiny_out", [P, 16], FP32, kind="ExternalOutput").ap()
    else:
        q = nc.dram_tensor("q", [T, B_LOC, NQ, D], FP32, kind="ExternalInput").ap()
        out = nc.dram_tensor("out", [T, B_LOC, NQ, D], FP32, kind="ExternalOutput").ap()
        dummy = None
    kvT = nc.dram_tensor("kvT", [D, T * B_LOC * NKV], FP32, kind="ExternalInput").ap()
    wgT = nc.dram_tensor("wgT", [D, D], FP32, kind="ExternalInput").ap()

    # [T, B, (nh p ns), d] -> [t, b, nh, p, (ns d)]
    q_v = q.rearrange("t b (nh p ns) d -> t b nh p (ns d)", nh=NH, p=P, ns=NSUB)
    out_v = out.rearrange("t b (nh p ns) d -> t b nh p (ns d)", nh=NH, p=P, ns=NSUB)
    # kvT rows: d = c*128 + p ; cols: i = t'*16 + b*4 + n
    kvT_v = kvT.rearrange("(c p) i -> p c i", p=P)
    wgT_v = wgT.rearrange("(c p) o -> p c o", p=P)
    NI = T * B_LOC * NKV  # 64
    NG = B_LOC * NKV      # 16 gate rows

    with tile.TileContext(nc) as tc:
        with (
            tc.tile_pool(name="const", bufs=1) as const_pool,
            tc.tile_pool(name="gbc", bufs=6) as gbc_pool,
            tc.tile_pool(name="qp", bufs=4) as q_pool,
            tc.tile_pool(name="cp", bufs=2) as c_pool,
            tc.tile_pool(name="ap", bufs=2) as a_pool,
            tc.tile_pool(name="wp", bufs=2) as w_pool,
            tc.tile_pool(name="sp", bufs=4) as s_pool,
            tc.tile_pool(name="psg", bufs=1, space="PSUM") as psg_pool,
            tc.tile_pool(name="gdram", bufs=1, space="DRAM") as gdram_pool,
        ):
            # ---- gate computation ----
            kvT_sb = const_pool.tile([P, 4 * NI], FP32, tag="kvT")
            nc.sync.dma_start(kvT_sb[:].rearrange("p (c i) -> p c i", c=4), kvT_v)
            wgT_sb = const_pool.tile([P, 4 * D], FP32, tag="wgT")
            nc.sync.dma_start(wgT_sb[:].rearrange("p (c o) -> p c o", c=4), wgT_v)

            # sum over t' of kvT (free layout per chunk: i = t'*16 + (b*4+n))
            kv4 = kvT_sb[:].rearrange("p (c tp i) -> p c tp i", c=4, tp=T)
            t01 = const_pool.tile([P, 4 * NG], FP32, tag="t01")
            t23 = const_pool.tile([P, 4 * NG], FP32, tag="t23")
            kvs = const_pool.tile([P, 4 * NG], FP32, tag="kvs")
            t01v = t01[:].rearrange("p (c i) -> p c i", c=4)
            t23v = t23[:].rearrange("p (c i) -> p c i", c=4)
            nc.vector.tensor_tensor(t01v, kv4[:, :, 0, :], kv4[:, :, 1, :], Alu.add)
            nc.vector.tensor_tensor(t23v, kv4[:, :, 2, :], kv4[:, :, 3, :], Alu.add)
            nc.vector.tensor_tensor(
                kvs[:].rearrange("p (c i) -> p c i", c=4), t01v, t23v, Alu.add
            )
            kvs_v = kvs[:].rearrange("p (c i) -> p c i", c=4)

            # scale Wg^T by 1/(2T) on DVE. Besides folding the mean and /tau into
            # the gate, this funnels both matmul operands through the DVE clock so
            # the PE load-weights slot needs only one sync wait (walrus limit).
            wgs = const_pool.tile([P, 4 * D], FP32, tag="wgs")
            nc.vector.tensor_scalar(
                wgs[:], wgT_sb[:], float(1.0 / (2.0 * T)), None, Alu.mult
            )
            wg_v = wgs[:].rearrange("p (c o) -> p c o", c=4)

            psum_g = psg_pool.tile([NG, D], FP32)
            for c in range(4):
                nc.tensor.matmul(psum_g[:], kvs_v[:, c, :], wg_v[:, c, :],
                                 start=(c == 0), stop=(c == 3))
            g_sb = const_pool.tile([NG, D], FP32, tag="gsb")
            nc.scalar.copy(g_sb[:], psum_g[:])

            # per-partition bias vector for the threshold sigmoid
            thr_bias = const_pool.tile([P, 1], FP32, tag="thrb")
            nc.vector.memset(thr_bias[:], -V_TH * _BIG)

            # round-trip the gate through DRAM so it can be partition-broadcast
            # on reload (engines cannot broadcast across partitions; DMA can).
            g_dram = gdram_pool.tile([NG, D], FP32, tag="gdram")
            nc.sync.dma_start(g_dram[:], g_sb[:])

            if timing_mode:
                # fill internal q with a constant so timing needs no transfer
                qfill = const_pool.tile([P, FREE], FP32, tag="qfill")
                nc.vector.memset(qfill[:], 0.3)
                for t in range(T):
                    for b in range(B_LOC):
                        for nh in range(NH):
                            nc.sync.dma_start(q_v[t, b, nh], qfill[:])
                nc.sync.dma_start(dummy, qfill[:, :16])  # satisfy external output

            import contextlib
            if timing_mode and repeats > 1:
                assert repeats % UNROLL == 0
                rep_ctx = tc.For_i(0, repeats // UNROLL, 1)
                inner_reps = UNROLL
            else:
                rep_ctx = contextlib.nullcontext()
                inner_reps = 1

            # ---- main loop ----
            with rep_ctx:
             for _inner in range(inner_reps):
              for b in range(B_LOC):
                # gate rows for this b, broadcast to all 128 partitions
                gbc = []
                for t in range(T):
                    row = b * NKV + t
                    gt = gbc_pool.tile([P, D], FP32, tag="gbc")
                    nc.sync.dma_start(
                        gt[:], g_dram[row:row + 1, :].partition_broadcast(P)
                    )
                    gbc.append(gt)

                for nh in range(NH):
                    w_prev = None
                    for t in range(T):
                        qt = q_pool.tile([P, FREE], FP32, tag="q")
                        nc.sync.dma_start(qt[:], q_v[t, b, nh])
                        ct = c_pool.tile([P, FREE], FP32, tag="c")
                        g_rep = gbc[t][:, None, :].to_broadcast((P, NSUB, D))
                        nc.vector.tensor_tensor(
                            ct[:].rearrange("p (ns d) -> p ns d", ns=NSUB),
                            qt[:].rearrange("p (ns d) -> p ns d", ns=NSUB),
                            g_rep, Alu.mult,
                        )
                        if t == 0:
                            at = ct
                        else:
                            at = a_pool.tile([P, FREE], FP32, tag="a")
                            nc.vector.scalar_tensor_tensor(
                                at[:], w_prev[:], 0.5, ct[:], Alu.mult, Alu.add
                            )
                        # s = (a >= 0.5) as exact 0.0/1.0: sigmoid saturates for
                        # |x| > ~17 and the ACT affine is a true fma, so the sign
                        # of BIG*(a-0.5) is exact; GPSIMD is_ge would be ~30us/op
                        # (Q7 slow path), ACT is ~2us.
                        st = s_pool.tile([P, FREE], FP32, tag="s")
                        nc.scalar.activation(
                            st[:], at[:], mybir.ActivationFunctionType.Sigmoid,
                            bias=thr_bias[:], scale=_BIG,
                        )
                        nc.scalar.dma_start(out_v[t, b, nh], st[:])
                        if t < T - 1:
                            wt = w_pool.tile([P, FREE], FP32, tag="w")
                            nc.vector.scalar_tensor_tensor(
                                wt[:], at[:], V_TH, at[:], Alu.is_lt, Alu.mult
                            )
                            w_prev = wt
    nc.compile()
    return nc


_CACHED_NC = None


def _make_in_maps(q, kv, Wg):
    q = np.ascontiguousarray(q, dtype=np.float32)
    kv = np.ascontiguousarray(kv, dtype=np.float32)
    Wg = np.ascontiguousarray(Wg, dtype=np.float32)

    # transposed so the contraction dim lands on partitions; the 1/(2T) gate
    # scaling (mean over T + LIF /tau) is applied on-device
    wgT = np.ascontiguousarray(Wg.T)

    in_maps = []
    for i in range(N_CORES):
        b0 = i * B_LOC
        q_i = np.ascontiguousarray(q[:, b0:b0 + B_LOC])
        kv_i = kv[:, b0:b0 + B_LOC]  # [T, B_LOC, NKV, D]
        kvT_i = np.ascontiguousarray(
            kv_i.transpose(3, 0, 1, 2).reshape(D, T * B_LOC * NKV)
        )
        in_maps.append({"q": q_i, "kvT": kvT_i, "wgT": wgT})
    return in_maps


def kernel(q: np.ndarray, kv: np.ndarray, Wg: np.ndarray) -> np.ndarray:
    global _CACHED_NC
    if _CACHED_NC is None:
        _CACHED_NC = build_kernel()
    nc = _CACHED_NC

    in_maps = _make_in_maps(q, kv, Wg)
    res = run_bass_kernel_spmd(nc, in_maps, core_ids=list(range(N_CORES)))
    out = np.concatenate([r["out"] for r in res.results], axis=1)
    return out


if __name__ == "__main__":
    rng = np.random.default_rng(0)
    q = rng.standard_normal((T, B, NQ, D), dtype=np.float32)
    kv = rng.standard_normal((T, B, NKV, D), dtype=np.float32)
    Wg = (rng.standard_normal((D, D), dtype=np.float32) / np.sqrt(D)).astype(np.float32)
    o = kernel(q, kv, Wg)
    print("out", o.shape, o.dtype, "mean", o.mean())


# revision 27
# speedup vs baseline: 6.2543x; 1.3431x over previous
"""Trainium2 Bass kernel for nn_MemoryUpdate (gated LIF memory update).

Reference computation (fp32):
    k         = einsum('tbnd,od->tbno', kv, Wg)          # kv @ Wg^T
    gate_mean = mean_t'( k[t', b, nkv, d] )              # [Nkv, B, 1, D], Nkv == T
    update    = gate_mean[t, b, d] * q[t, b, n, d]       # broadcast over n
    spikes    = LIF over t: v' = (v + u)/2 ; s = v' >= 0.5 ; v = v' * (1 - s)

Shapes: q [4, 32, 1024, 512], kv [4, 32, 4, 512], Wg [512, 512] -> out [4, 32, 1024, 512].

Strategy: data-parallel over B across 8 cores (B_loc = 4). Per core:
  - tiny on-device matmul for the gate:   g'[b*4+t, o] = sum_d kvsumT[d, b*4+t] * WgT[d, o]
    where WgT is host-side-transposed Wg scaled by 1/(2T)  (folds the mean and the
    LIF /tau into the gate), and kvsumT is the T-sum of host-transposed kv.
  - gate rows broadcast to [128, 2048] tiles via PE outer-product (ones ⊗ g_row).
  - main stream, tiles [128 part = n/4, free = (n_sub 4, d 512)] (8 KiB contiguous
    per partition per DMA):
        c = q * g_bc                  (DVE tensor_tensor)
        a = (w*0.5) + c               (DVE scalar_tensor_tensor; skipped at t=0)
        s = (a >= 0.5)                (GpSimd tensor_scalar)  -> store
        w = (a < 0.5) * a             (DVE scalar_tensor_tensor; skipped at last t)
Recurrence over t kept in SBUF; (b, n)-independent => fully pipelined.
"""

import sys

for p in ("/opt/trn_rl_repo", "/root/.axon_site/_ro/trn_rl_repo"):
    if p not in sys.path:
        sys.path.insert(0, p)

import numpy as np

import concourse.bass as bass
import concourse.mybir as mybir
import concourse.tile as tile
from concourse import bacc
from concourse.bass_utils import run_bass_kernel_spmd

# Problem constants (hardcoded per harness contract)
T, B, NQ, NKV, D = 4, 32, 1024, 4, 512
N_CORES = 8
B_LOC = B // N_CORES  # 4
V_TH = 0.5
NSUB = 8              # n-values packed per partition row
P = 128               # partitions
FREE = NSUB * D       # 4096 (one [128, 4096] tile covers all of Nq for one b)

FP32 = mybir.dt.float32
Alu = mybir.AluOpType
UNROLL = 8  # static inner unroll inside the timing-mode For_i loop
_BIG = 1.0e30  # threshold-comparison scale; saturates sigmoid to exact 0/1


def build_kernel(repeats=1, timing_mode=False):
    # Bacc (not raw Bass): its compile() legalizes multi-sem waits, which the
    # walrus CoreV3 codegen can't carry on a single compute instruction.
    nc = bacc.Bacc("TRN2", target_bir_lowering=False, debug=False,
                   num_devices=N_CORES)

    if timing_mode:
        # timing-only variant: big tensors live in internal DRAM so the wall
        # clock isn't dominated by host<->device transfers; the main body runs
        # `repeats` times in an on-device loop.
        q = nc.dram_tensor("q_int", [T, B_LOC, NQ, D], FP32).ap()
        out = nc.dram_tensor("out_int", [T, B_LOC, NQ, D], FP32).ap()
        dummy = nc.dram_tensor("tiny_out", [P, 16], FP32, kind="ExternalOutput").ap()
    else:
        q = nc.dram_tensor("q", [T, B_LOC, NQ, D], FP32, kind="ExternalInput").ap()
        out = nc.dram_tensor("out", [T, B_LOC, NQ, D], FP32, kind="ExternalOutput").ap()
        dummy = None
    kvT = nc.dram_tensor("kvT", [D, T * B_LOC * NKV], FP32, kind="ExternalInput").ap()
    wgT = nc.dram_tensor("wgT", [D, D], FP32, kind="ExternalInput").ap()

    # [T, B, (p ns), d] -> [t, b, p, (ns d)]: one [128, 4096] tile per (t, b),
    # 16 KiB contiguous per partition per DMA (2 MiB transfers: ~336 GB/s
    # measured vs ~286 GB/s at 1 MiB).
    q_v = q.rearrange("t b (p ns) d -> t b p (ns d)", p=P, ns=NSUB)
    out_v = out.rearrange("t b (p ns) d -> t b p (ns d)", p=P, ns=NSUB)
    # kvT rows: d = c*128 + p ; cols: i = t'*16 + b*4 + n
    kvT_v = kvT.rearrange("(c p) i -> p c i", p=P)
    wgT_v = wgT.rearrange("(c p) o -> p c o", p=P)
    NI = T * B_LOC * NKV  # 64
    NG = B_LOC * NKV      # 16 gate rows

    with tile.TileContext(nc) as tc:
        with (
            tc.tile_pool(name="const", bufs=1) as const_pool,
            tc.tile_pool(name="gbc", bufs=6) as gbc_pool,
            tc.tile_pool(name="qp", bufs=3) as q_pool,
            tc.tile_pool(name="cp", bufs=3) as c_pool,
            tc.tile_pool(name="wp", bufs=2) as w_pool,
            tc.tile_pool(name="sp", bufs=2) as s_pool,
            tc.tile_pool(name="psg", bufs=1, space="PSUM") as psg_pool,
            tc.tile_pool(name="gdram", bufs=1, space="DRAM") as gdram_pool,
        ):
            # ---- gate computation ----
            kvT_sb = const_pool.tile([P, 4 * NI], FP32, tag="kvT")
            nc.sync.dma_start(kvT_sb[:].rearrange("p (c i) -> p c i", c=4), kvT_v)
            wgT_sb = const_pool.tile([P, 4 * D], FP32, tag="wgT")
            nc.sync.dma_start(wgT_sb[:].rearrange("p (c o) -> p c o", c=4), wgT_v)

            # sum over t' of kvT (free layout per chunk: i = t'*16 + (b*4+n))
            kv4 = kvT_sb[:].rearrange("p (c tp i) -> p c tp i", c=4, tp=T)
            t01 = const_pool.tile([P, 4 * NG], FP32, tag="t01")
            t23 = const_pool.tile([P, 4 * NG], FP32, tag="t23")
            kvs = const_pool.tile([P, 4 * NG], FP32, tag="kvs")
            t01v = t01[:].rearrange("p (c i) -> p c i", c=4)
            t23v = t23[:].rearrange("p (c i) -> p c i", c=4)
            nc.vector.tensor_tensor(t01v, kv4[:, :, 0, :], kv4[:, :, 1, :], Alu.add)
            nc.vector.tensor_tensor(t23v, kv4[:, :, 2, :], kv4[:, :, 3, :], Alu.add)
            nc.vector.tensor_tensor(
                kvs[:].rearrange("p (c i) -> p c i", c=4), t01v, t23v, Alu.add
            )
            kvs_v = kvs[:].rearrange("p (c i) -> p c i", c=4)

            # scale Wg^T by 1/(2T) on DVE. Besides folding the mean and /tau into
            # the gate, this funnels both matmul operands through the DVE clock so
            # the PE load-weights slot needs only one sync wait (walrus limit).
            wgs = const_pool.tile([P, 4 * D], FP32, tag="wgs")
            nc.vector.tensor_scalar(
                wgs[:], wgT_sb[:], float(1.0 / (2.0 * T)), None, Alu.mult
            )
            wg_v = wgs[:].rearrange("p (c o) -> p c o", c=4)

            psum_g = psg_pool.tile([NG, D], FP32)
            for c in range(4):
                nc.tensor.matmul(psum_g[:], kvs_v[:, c, :], wg_v[:, c, :],
                                 start=(c == 0), stop=(c == 3))
            g_sb = const_pool.tile([NG, D], FP32, tag="gsb")
            nc.scalar.copy(g_sb[:], psum_g[:])

            # per-partition bias vector for the threshold sigmoid
            thr_bias = const_pool.tile([P, 1], FP32, tag="thrb")
            nc.vector.memset(thr_bias[:], -V_TH * _BIG)

            # round-trip the gate through DRAM so it can be partition-broadcast
            # on reload (engines cannot broadcast across partitions; DMA can).
            g_dram = gdram_pool.tile([NG, D], FP32, tag="gdram")
            nc.sync.dma_start(g_dram[:], g_sb[:])

            if timing_mode:
                # fill internal q with a constant so timing needs no transfer
                qfill = const_pool.tile([P, FREE], FP32, tag="qfill")
                nc.vector.memset(qfill[:], 0.3)
                for t in range(T):
                    for b in range(B_LOC):
                        nc.sync.dma_start(q_v[t, b], qfill[:])
                nc.sync.dma_start(dummy, qfill[:, :16])  # satisfy external output

            import contextlib
            if timing_mode and repeats > 1:
                assert repeats % UNROLL == 0
                rep_ctx = tc.For_i(0, repeats // UNROLL, 1)
                inner_reps = UNROLL
            else:
                rep_ctx = contextlib.nullcontext()
                inner_reps = 1

            # ---- main loop ----
            with rep_ctx:
             for _inner in range(inner_reps):
              for b in range(B_LOC):
                # gate rows for this b, broadcast to all 128 partitions
                gbc = []
                for t in range(T):
                    row = b * NKV + t
                    gt = gbc_pool.tile([P, D], FP32, tag="gbc")
                    nc.sync.dma_start(
                        gt[:], g_dram[row:row + 1, :].partition_broadcast(P)
                    )
                    gbc.append(gt)

                w_prev = None
                for t in range(T):
                    qt = q_pool.tile([P, FREE], FP32, tag="q")
                    nc.sync.dma_start(qt[:], q_v[t, b])
                    ct = c_pool.tile([P, FREE], FP32, tag="c")
                    g_rep = gbc[t][:, None, :].to_broadcast((P, NSUB, D))
                    nc.vector.tensor_tensor(
                        ct[:].rearrange("p (ns d) -> p ns d", ns=NSUB),
                        qt[:].rearrange("p (ns d) -> p ns d", ns=NSUB),
                        g_rep, Alu.mult,
                    )
                    at = ct
                    if t > 0:
                        # a = w/2 + c, in place over the c tile
                        nc.vector.scalar_tensor_tensor(
                            at[:], w_prev[:], 0.5, ct[:], Alu.mult, Alu.add
                        )
                    # s = (a >= 0.5) as exact 0.0/1.0: sigmoid saturates for
                    # |x| > ~17 and the ACT affine is a true fma, so the sign
                    # of BIG*(a-0.5) is exact; GPSIMD is_ge would be ~30us/op
                    # (Q7 slow path), ACT is ~2us.
                    st = s_pool.tile([P, FREE], FP32, tag="s")
                    nc.scalar.activation(
                        st[:], at[:], mybir.ActivationFunctionType.Sigmoid,
                        bias=thr_bias[:], scale=_BIG,
                    )
                    nc.scalar.dma_start(out_v[t, b], st[:])
                    if t < T - 1:
                        wt = w_pool.tile([P, FREE], FP32, tag="w")
                        nc.vector.scalar_tensor_tensor(
                            wt[:], at[:], V_TH, at[:], Alu.is_lt, Alu.mult
                        )
                        w_prev = wt
    nc.compile()
    return nc


_CACHED_NC = None


def _make_in_maps(q, kv, Wg):
    q = np.ascontiguousarray(q, dtype=np.float32)
    kv = np.ascontiguousarray(kv, dtype=np.float32)
    Wg = np.ascontiguousarray(Wg, dtype=np.float32)

    # transposed so the contraction dim lands on partitions; the 1/(2T) gate
    # scaling (mean over T + LIF /tau) is applied on-device
    wgT = np.ascontiguousarray(Wg.T)

    in_maps = []
    for i in range(N_CORES):
        b0 = i * B_LOC
        q_i = np.ascontiguousarray(q[:, b0:b0 + B_LOC])
        kv_i = kv[:, b0:b0 + B_LOC]  # [T, B_LOC, NKV, D]
        kvT_i = np.ascontiguousarray(
            kv_i.transpose(3, 0, 1, 2).reshape(D, T * B_LOC * NKV)
        )
        in_maps.append({"q": q_i, "kvT": kvT_i, "wgT": wgT})
    return in_maps


def kernel(q: np.ndarray, kv: np.ndarray, Wg: np.ndarray) -> np.ndarray:
    global _CACHED_NC
    if _CACHED_NC is None:
        _CACHED_NC = build_kernel()
    nc = _CACHED_NC

    in_maps = _make_in_maps(q, kv, Wg)
    res = run_bass_kernel_spmd(nc, in_maps, core_ids=list(range(N_CORES)))
    out = np.concatenate([r["out"] for r in res.results], axis=1)
    return out


if __name__ == "__main__":
    rng = np.random.default_rng(0)
    q = rng.standard_normal((T, B, NQ, D), dtype=np.float32)
    kv = rng.standard_normal((T, B, NKV, D), dtype=np.float32)
    Wg = (rng.standard_normal((D, D), dtype=np.float32) / np.sqrt(D)).astype(np.float32)
    o = kernel(q, kv, Wg)
    print("out", o.shape, o.dtype, "mean", o.mean())
